# revision 1
# baseline (speedup 1.0000x reference)
"""DLRM (nn_DLRM_RPC) Trainium2 Bass kernel.

Strategy: pure data-parallel over batch across 8 NeuronCores; embedding
tables replicated in each core's HBM (bf16), so no collectives.

Per core (2048 samples, 4 sample-tiles of 512):
  - one multi-index indirect DMA per 128-sample chunk gathers all 26
    embedding rows per sample (bf16, host-precast) into SBUF sample-major
  - PE transposes flip the gathered vectors feature-major into a
    sample-major grouped layout Eg[d, 32*sample + t] (t = slot: 0 =
    bottom-MLP output x, 1..26 = embeddings, 27..31 pad).  Because
    32*(4g+s) + t = 128g + 32s + t, each group of 4 samples occupies one
    contiguous 128-column block while per-table transpose drains stay
    single-strided (stride 32) - no slow multi-axis scatter.
  - per-group Gram matmuls B_g = blk^T @ blk on the contiguous group
    block give all 27x27 interaction dot products for 4 samples at once
    (rows/cols 32s+t); drains to zsb alternate vector/scalar
  - a merged scramble pass (16 copies per tile, one per (u, s), with
    32-aligned partition bases) rearranges Z into 7 K-stacked tiles
    zstk[32u+j, 512q + 4g + s] = Z_{4g+s}[4q+u, j], spread across
    vector/gpsimd/scalar
  - top MLP consumes [x ; zstk] with host-rearranged W0 (symmetric Z
    means only lower-triangle weights are placed), ReLU/Sigmoid fused
    into the PSUM->SBUF drains on the scalar engine
  - phases are software-pipelined: tile n+1's gather/transpose/bottom
    MLP are emitted between tile n's Gram and top MLP so the PE array
    stays busy while the scramble copies run.

All matmuls bf16 with fp32 PSUM accumulation.
"""

import os
import sys

import numpy as np

for _p in ("/opt/trn_rl_repo",):
    if _p not in sys.path and os.path.isdir(_p):
        sys.path.insert(0, _p)

import ml_dtypes

import concourse.bass as bass
import concourse.bacc as bacc
import concourse.mybir as mybir
import concourse.tile as tile
from concourse import bass_utils
from concourse.bass_interp import get_hw_module
from concourse.masks import make_identity

BF16 = ml_dtypes.bfloat16
F32 = np.float32

N_CORES = 8
B = 16384
SPC = B // N_CORES        # samples per core: 2048
NT = 27                   # slots: x + 26 tables
NE = 26
VOCAB = 50000
D = 128
TS = 512                  # samples per tile
NTILES = SPC // TS        # 4
G = TS // 4               # groups per tile: 128
CH = TS // 128            # 128-sample chunks per tile: 4
BW = 128                  # group block width: 32 * 4

LI, LJ = np.tril_indices(NT, -1)

_dt_bf16 = mybir.dt.bfloat16
_dt_f32 = mybir.dt.float32
_dt_i32 = mybir.dt.int32

_CACHE = {}


def _emit(tc, t):
    from contextlib import ExitStack

    nc = tc.nc
    Relu = mybir.ActivationFunctionType.Relu
    Sigmoid = mybir.ActivationFunctionType.Sigmoid

    with ExitStack() as ctx:
        sb = ctx.enter_context(tc.tile_pool(name="sb", bufs=1))
        db = ctx.enter_context(tc.tile_pool(name="db", bufs=2))
        sp = ctx.enter_context(tc.tile_pool(name="sp", bufs=1))
        mmps = ctx.enter_context(tc.tile_pool(name="mmps", bufs=2, space="PSUM"))
        grps = ctx.enter_context(tc.tile_pool(name="grps", bufs=3, space="PSUM"))
        trps = ctx.enter_context(tc.tile_pool(name="trps", bufs=2, space="PSUM"))

        ident = sb.tile([128, 128], _dt_bf16)
        make_identity(nc, ident[:])

        # --- load weights/inputs that stay resident ---
        def load(name, shape, dtype=_dt_bf16):
            tl = sb.tile(shape, dtype, name=name)
            nc.sync.dma_start(tl[:], t[name][:])
            return tl

        dxt = load("dxt", [16, SPC])
        bw0 = load("bw0", [16, 512])
        bb0 = load("bb0", [128, 4], _dt_f32)
        bw1 = load("bw1", [128, 4 * 256])
        bb1 = load("bb1", [128, 2], _dt_f32)
        bw2 = load("bw2", [128, 2 * 128])
        bb2 = load("bb2", [128, 1], _dt_f32)
        w0x = load("w0x", [128, 1024])
        wz = load("wz", [128, 7 * 1024])
        tb0 = load("tb0", [128, 8], _dt_f32)
        w1 = load("w1", [128, 8 * 1024])
        tb1 = load("tb1", [128, 8], _dt_f32)
        w2 = load("w2", [128, 8 * 512])
        tb2 = load("tb2", [128, 4], _dt_f32)
        w3 = load("w3", [128, 4])
        tb3 = load("tb3", [1, 1], _dt_f32)

        eg0 = sb.tile([128, BW * G], _dt_bf16, name="eg0")
        eg1 = sb.tile([128, BW * G], _dt_bf16, name="eg1")
        egs = [eg0, eg1, eg0, eg1]
        zsb = sb.tile([128, BW * G], _dt_bf16)
        zstk = sb.tile([128, 7 * TS], _dt_bf16)
        nc.vector.memset(zstk[:], 0.0)

        # gpsimd cannot access PSUM: PSUM->SBUF drains alternate
        # vector/scalar; the SBUF->SBUF scramble leans on gpsimd.
        drain_engines = [nc.vector, nc.scalar]
        scram_engines = [nc.gpsimd, nc.gpsimd, nc.vector, nc.scalar]

        def cp(engine, dst, src):
            if engine is nc.scalar:
                engine.copy(dst, src)
            else:
                engine.tensor_copy(dst, src)

        def phase_a(n, eg):
            """gather + transpose tile n into eg (col = 32*sample + t)."""
            ef = eg[:]
            for c in range(CH):
                C = CH * n + c
                idxt = db.tile([128, NE], _dt_i32, name="idxt")
                nc.sync.dma_start(idxt[:], t["idx"][128 * C:128 * (C + 1), :])
                esm = db.tile([128, NE * D], _dt_bf16, name="esm")
                nc.gpsimd.indirect_dma_start(
                    out=esm[:], out_offset=None,
                    in_=t["tbl"][:],
                    in_offset=bass.IndirectOffsetOnAxis(ap=idxt[:], axis=0),
                )
                for t8 in range(4):
                    nt8 = 8 if t8 < 3 else 2
                    trp = trps.tile([128, 128 * 8], _dt_bf16,
                                    name="trp", tag="trp")
                    for k in range(nt8):
                        ti = 8 * t8 + k
                        nc.tensor.transpose(
                            trp[:, 128 * k:128 * (k + 1)],
                            esm[:, 128 * ti:128 * (ti + 1)], ident[:])
                    dst = bass.AP(ef.tensor,
                                  ef.offset + BW * 32 * c + (8 * t8 + 1),
                                  [ef.ap[0], [1, nt8], [32, 128]])
                    eng = drain_engines[(c + t8) % 2]
                    cp(eng, dst, trp[:, 0:128 * nt8])

        def phase_b(n, eg):
            """bottom MLP -> x into eg slot 0."""
            h0 = sp.tile([128, 4 * 512], _dt_bf16, name="h0")
            for m in range(4):
                ps = mmps.tile([128, 512], _dt_f32, name="mm", tag="mm")
                nc.tensor.matmul(ps[:], bw0[:, 128 * m:128 * (m + 1)],
                                 dxt[:, TS * n:TS * (n + 1)],
                                 start=True, stop=True)
                nc.scalar.activation(h0[:, 512 * m:512 * (m + 1)], ps[:],
                                     Relu, bias=bb0[:, m:m + 1])
            h1b = sp.tile([128, 2 * 512], _dt_bf16, name="h1b")
            for m in range(2):
                ps = mmps.tile([128, 512], _dt_f32, name="mm", tag="mm")
                for k in range(4):
                    nc.tensor.matmul(
                        ps[:], bw1[:, 256 * k + 128 * m:256 * k + 128 * (m + 1)],
                        h0[:, 512 * k:512 * (k + 1)],
                        start=(k == 0), stop=(k == 3))
                nc.scalar.activation(h1b[:, 512 * m:512 * (m + 1)], ps[:],
                                     Relu, bias=bb1[:, m:m + 1])
            ps = mmps.tile([128, 512], _dt_f32, name="mm", tag="mm")
            for k in range(2):
                nc.tensor.matmul(ps[:], bw2[:, 128 * k:128 * (k + 1)],
                                 h1b[:, 512 * k:512 * (k + 1)],
                                 start=(k == 0), stop=(k == 1))
            ef = eg[:]
            xdst = bass.AP(ef.tensor, ef.offset, [ef.ap[0], [32, 512]])
            nc.scalar.activation(xdst, ps[:], Relu, bias=bb2[:, 0:1])

        def phase_c(n, eg):
            """Gram matmuls into zsb (rows 32s+t, col-in-group 32s'+t')."""
            for r in range(G // 4):
                bank = grps.tile([128, 512], _dt_f32, name="grb", tag="gr")
                for k in range(4):
                    g = 4 * r + k
                    blk = eg[:, BW * g:BW * (g + 1)]
                    nc.tensor.matmul(bank[:, 128 * k:128 * (k + 1)],
                                     blk, blk, start=True, stop=True)
                eng = drain_engines[r % 2]
                cp(eng, zsb[:, 512 * r:512 * (r + 1)], bank[:])

        def phase_d(n):
            """merged scramble: zstk[32u+j, 512q+4g+s] = Z_{4g+s}[4q+u, j]."""
            for e, (u, s) in enumerate((u, s) for u in range(4)
                                       for s in range(4)):
                nq = 7 if u < 3 else 6   # i = 4q+u must stay < 27
                sl = zsb[32 * s:32 * s + NT, :]
                src = bass.AP(sl.tensor, sl.offset + 32 * s + u,
                              [sl.ap[0], [4, nq], [BW, G]])
                dl = zstk[32 * u:32 * u + NT, :]
                dst = bass.AP(dl.tensor, dl.offset + s,
                              [dl.ap[0], [512, nq], [4, G]])
                cp(scram_engines[e % 4], dst, src)

        def phase_e(n, eg):
            """top MLP on [x ; zstk]."""
            ef = eg[:]
            xap = bass.AP(ef.tensor, ef.offset, [ef.ap[0], [32, 512]])
            h1t = sp.tile([128, 8 * 512], _dt_bf16, name="h1t")
            for m in range(8):
                ps = mmps.tile([128, 512], _dt_f32, name="mm", tag="mm")
                nc.tensor.matmul(ps[:], w0x[:, 128 * m:128 * (m + 1)], xap,
                                 start=True, stop=False)
                for q in range(7):
                    nc.tensor.matmul(
                        ps[:], wz[:, 1024 * q + 128 * m:1024 * q + 128 * (m + 1)],
                        zstk[:, 512 * q:512 * (q + 1)],
                        start=False, stop=(q == 6))
                nc.scalar.activation(h1t[:, 512 * m:512 * (m + 1)], ps[:],
                                     Relu, bias=tb0[:, m:m + 1])
            h2t = sp.tile([128, 8 * 512], _dt_bf16, name="h2t")
            for m in range(8):
                ps = mmps.tile([128, 512], _dt_f32, name="mm", tag="mm")
                for k in range(8):
                    nc.tensor.matmul(
                        ps[:], w1[:, 1024 * k + 128 * m:1024 * k + 128 * (m + 1)],
                        h1t[:, 512 * k:512 * (k + 1)],
                        start=(k == 0), stop=(k == 7))
                nc.scalar.activation(h2t[:, 512 * m:512 * (m + 1)], ps[:],
                                     Relu, bias=tb1[:, m:m + 1])
            h3t = sp.tile([128, 4 * 512], _dt_bf16, name="h3t")
            for m in range(4):
                ps = mmps.tile([128, 512], _dt_f32, name="mm", tag="mm")
                for k in range(8):
                    nc.tensor.matmul(
                        ps[:], w2[:, 512 * k + 128 * m:512 * k + 128 * (m + 1)],
                        h2t[:, 512 * k:512 * (k + 1)],
                        start=(k == 0), stop=(k == 7))
                nc.scalar.activation(h3t[:, 512 * m:512 * (m + 1)], ps[:],
                                     Relu, bias=tb2[:, m:m + 1])
            ps3 = mmps.tile([128, 512], _dt_f32, name="mm", tag="mm")
            for k in range(4):
                nc.tensor.matmul(ps3[0:1, :], w3[:, k:k + 1],
                                 h3t[:, 512 * k:512 * (k + 1)],
                                 start=(k == 0), stop=(k == 3))
            outsb = db.tile([1, 512], _dt_f32, name="outsb")
            nc.scalar.activation(outsb[:], ps3[0:1, :], Sigmoid,
                                 bias=tb3[0:1, 0:1])
            nc.sync.dma_start(t["out"][n:n + 1, :], outsb[:])

        # software pipeline: A/B of tile n+1 slot between C/D and E of tile n
        phase_a(0, egs[0])
        phase_b(0, egs[0])
        for n in range(NTILES):
            phase_c(n, egs[n])
            phase_d(n)
            if n + 1 < NTILES:
                phase_a(n + 1, egs[n + 1])
                phase_b(n + 1, egs[n + 1])
            phase_e(n, egs[n])


def _build():
    if "nc" in _CACHE:
        return _CACHE["nc"]
    nc = bacc.Bacc("TRN2", target_bir_lowering=False, debug=False,
                   num_devices=N_CORES)
    t = {}

    def dram(name, shape, dt, kind="ExternalInput"):
        t[name] = nc.dram_tensor(name, shape, dt, kind=kind).ap()

    dram("tbl", [NE * VOCAB, D], _dt_bf16)
    dram("idx", [SPC, NE], _dt_i32)
    dram("dxt", [16, SPC], _dt_bf16)
    dram("bw0", [16, 512], _dt_bf16)
    dram("bb0", [128, 4], _dt_f32)
    dram("bw1", [128, 4 * 256], _dt_bf16)
    dram("bb1", [128, 2], _dt_f32)
    dram("bw2", [128, 2 * 128], _dt_bf16)
    dram("bb2", [128, 1], _dt_f32)
    dram("w0x", [128, 1024], _dt_bf16)
    dram("wz", [128, 7 * 1024], _dt_bf16)
    dram("tb0", [128, 8], _dt_f32)
    dram("w1", [128, 8 * 1024], _dt_bf16)
    dram("tb1", [128, 8], _dt_f32)
    dram("w2", [128, 8 * 512], _dt_bf16)
    dram("tb2", [128, 4], _dt_f32)
    dram("w3", [128, 4], _dt_bf16)
    dram("tb3", [1, 1], _dt_f32)
    dram("out", [NTILES, TS], _dt_f32, kind="ExternalOutput")

    with tile.TileContext(nc) as tc:
        _emit(tc, t)
    nc.compile()

    _CACHE["nc"] = nc
    return nc


def _ktile(w, kt, m):
    """[K, M] -> [128, (K//128) * M] with column kt*M + mm = w[128*kt + p, mm]."""
    K, Mo = w.shape
    return np.ascontiguousarray(
        w.reshape(K // 128, 128, Mo).transpose(1, 0, 2).reshape(128, -1))


def _shared_inputs(inputs):
    emb = np.asarray(inputs["emb_tables"])
    tbl = np.ascontiguousarray(
        emb.astype(BF16).reshape(NE * VOCAB, D))

    sh = {"tbl": tbl}
    sh["bw0"] = np.zeros((16, 512), BF16)
    sh["bw0"][:13] = np.asarray(inputs["bot_W0"]).astype(BF16)
    sh["bb0"] = np.asarray(inputs["bot_b0"]).astype(F32).reshape(4, 128).T.copy()
    sh["bw1"] = _ktile(np.asarray(inputs["bot_W1"]).astype(BF16), 4, 256)
    sh["bb1"] = np.asarray(inputs["bot_b1"]).astype(F32).reshape(2, 128).T.copy()
    sh["bw2"] = _ktile(np.asarray(inputs["bot_W2"]).astype(BF16), 2, 128)
    sh["bb2"] = np.asarray(inputs["bot_b2"]).astype(F32).reshape(1, 128).T.copy()

    w0 = np.asarray(inputs["top_W0"]).astype(F32)
    sh["w0x"] = w0[:128].astype(BF16)
    wgrid = np.zeros((NT, NT, 1024), F32)
    wgrid[LI, LJ] = w0[128:479]
    wz4 = np.zeros((7, 128, 1024), F32)
    for i in range(NT):
        q, u = i // 4, i % 4
        wz4[q, 32 * u:32 * u + NT] = wgrid[i]
    sh["wz"] = np.ascontiguousarray(
        wz4.transpose(1, 0, 2).reshape(128, 7 * 1024)).astype(BF16)
    sh["tb0"] = np.asarray(inputs["top_b0"]).astype(F32).reshape(8, 128).T.copy()
    sh["w1"] = _ktile(np.asarray(inputs["top_W1"]).astype(BF16), 8, 1024)
    sh["tb1"] = np.asarray(inputs["top_b1"]).astype(F32).reshape(8, 128).T.copy()
    sh["w2"] = _ktile(np.asarray(inputs["top_W2"]).astype(BF16), 8, 512)
    sh["tb2"] = np.asarray(inputs["top_b2"]).astype(F32).reshape(4, 128).T.copy()
    sh["w3"] = _ktile(np.asarray(inputs["top_W3"]).astype(BF16), 4, 1)
    sh["tb3"] = np.asarray(inputs["top_b3"]).astype(F32).reshape(1, 1)
    return sh


def _in_maps(inputs):
    sh = _shared_inputs(inputs)
    idx = np.asarray(inputs["indices"]).astype(np.int64)      # [26, B]
    gidx = (idx + (np.arange(NE) * VOCAB)[:, None]).astype(np.int32)
    dx = np.asarray(inputs["dense_x"]).astype(F32)            # [B, 13]
    maps = []
    for core in range(N_CORES):
        sl = slice(SPC * core, SPC * (core + 1))
        m = dict(sh)
        m["idx"] = np.ascontiguousarray(gidx[:, sl].T)        # [2048, 26]
        dxt = np.zeros((16, SPC), BF16)
        dxt[:13] = dx[sl].T.astype(BF16)
        m["dxt"] = dxt
        maps.append(m)
    return maps


def _run(inputs, trace=False):
    nc = _build()
    maps = _in_maps(inputs)
    old_m = nc.m
    nc.m = _CACHE.setdefault("hwm", get_hw_module(nc.m))
    try:
        res = bass_utils.run_bass_kernel_spmd(
            nc, maps, core_ids=list(range(N_CORES)), trace=trace)
    finally:
        nc.m = old_m
    out = np.concatenate([r["out"].reshape(-1) for r in res.results])
    return out.astype(F32).reshape(B, 1), res


def kernel(**inputs):
    out, _ = _run(inputs, trace=False)
    return out



# revision 4
# speedup vs baseline: 1.0609x; 1.0609x over previous
"""DLRM (nn_DLRM_RPC) Trainium2 Bass kernel — fp8 DoubleRow edition.

Strategy: pure data-parallel over batch across 8 NeuronCores; embedding
tables replicated in each core's HBM (fp8 e4m3, host-precast with a x16
scale), so no collectives.

Per core (2048 samples, 4 sample-tiles of 512):
  - one multi-index indirect DMA per 128-sample chunk gathers all 26
    embedding rows per sample (fp8) into SBUF sample-major
  - PE transposes flip the gathered vectors feature-major into a
    sample-major grouped layout Eg[d, 32*sample + t] (t = slot: 0 =
    bottom-MLP output x scaled by S0, 1..26 = embeddings scaled by 16).
  - per-group Gram matmuls B_g = blk^T @ blk (fp8 operands, fp32 PSUM)
    give all 27x27 interaction dot products for 4 samples at once,
    scaled per-pair by s_i*s_j; drains cast straight to fp8 zsb
    (max |Z~| ~ 6 << 240, no saturation)
  - a merged scramble pass rearranges Z into 7 K-stacked fp8 tiles
    zstk[32u+j, 512(q+1) + 4g + s] = Z~_{4g+s}[4q+u, j]; zstk block 0
    holds 16*x written contiguously by the bottom MLP
  - top MLP L0/L1/L2 run as fp8 DoubleRow matmuls (two 128-K subtiles
    per instruction at double rate); per-pair scale compensation is
    folded into the host-side weight rearrangement, per-layer 1/S
    scales folded into the activation drains. L3 (logit) stays bf16.
  - all PSUM->SBUF drains and the scramble alternate scalar/vector
    (gpsimd copies are ~5x slower, so gpsimd only triggers gathers)
  - phases are software-pipelined: tile n+1's gather/transpose/bottom
    MLP are emitted between tile n's Gram and top MLP.
"""

import os
import sys

import numpy as np

for _p in ("/opt/trn_rl_repo",):
    if _p not in sys.path and os.path.isdir(_p):
        sys.path.insert(0, _p)

import ml_dtypes

import concourse.bass as bass
import concourse.bacc as bacc
import concourse.mybir as mybir
import concourse.tile as tile
from concourse import bass_utils
from concourse.bass_interp import get_hw_module
from concourse.masks import make_identity

BF16 = ml_dtypes.bfloat16
F8 = ml_dtypes.float8_e4m3
F32 = np.float32

N_CORES = 8
B = 16384
SPC = B // N_CORES        # samples per core: 2048
NT = 27                   # slots: x + 26 tables
NE = 26
VOCAB = 50000
D = 128
TS = 512                  # samples per tile
NTILES = SPC // TS        # 4
G = TS // 4               # groups per tile: 128
CH = TS // 128            # 128-sample chunks per tile: 4
BW = 128                  # group block width: 32 * 4

SE = 16.0                 # embedding-table scale (host precast)
S0 = 0.5                  # x slot scale inside the Gram
SX = 16.0                 # x scale in zstk block 0
SZ = 256.0                # common L0 PSUM scale target
SH = 8.0                  # h1/h2 activation scale

LI, LJ = np.tril_indices(NT, -1)

_dt_bf16 = mybir.dt.bfloat16
_dt_f8 = mybir.dt.float8e4
_dt_f32 = mybir.dt.float32
_dt_i32 = mybir.dt.int32

_CACHE = {}


def _emit(tc, t):
    from contextlib import ExitStack

    nc = tc.nc
    Relu = mybir.ActivationFunctionType.Relu
    Sigmoid = mybir.ActivationFunctionType.Sigmoid
    DR = mybir.MatmulPerfMode.DoubleRow

    with ExitStack() as ctx:
        sb = ctx.enter_context(tc.tile_pool(name="sb", bufs=1))
        db = ctx.enter_context(tc.tile_pool(name="db", bufs=2))
        sp = ctx.enter_context(tc.tile_pool(name="sp", bufs=1))
        mmps = ctx.enter_context(tc.tile_pool(name="mmps", bufs=2, space="PSUM"))
        grps = ctx.enter_context(tc.tile_pool(name="grps", bufs=3, space="PSUM"))
        trps = ctx.enter_context(tc.tile_pool(name="trps", bufs=2, space="PSUM"))

        ident = sb.tile([128, 128], _dt_f8)
        make_identity(nc, ident[:])

        # --- load weights/inputs that stay resident ---
        def load(name, shape, dtype=_dt_bf16):
            tl = sb.tile(shape, dtype, name=name)
            nc.sync.dma_start(tl[:], t[name][:])
            return tl

        dxt = load("dxt", [16, SPC])
        bw0 = load("bw0", [16, 512])
        bb0 = load("bb0", [128, 4], _dt_f32)
        bw1 = load("bw1", [128, 4 * 256])
        bb1 = load("bb1", [128, 2], _dt_f32)
        bw2 = load("bw2", [128, 2 * 128])
        bb2h = load("bb2h", [128, 1], _dt_f32)   # 0.5 * b2 (slot-0 write)
        bb2x = load("bb2x", [128, 1], _dt_f32)   # 16  * b2 (zstk block-0)
        wcat = load("wcat", [128, 8 * 1024], _dt_f8)
        tb0 = load("tb0", [128, 8], _dt_f32)
        w1 = load("w1", [128, 8 * 1024], _dt_f8)
        tb1 = load("tb1", [128, 8], _dt_f32)
        w2 = load("w2", [128, 8 * 512], _dt_f8)
        tb2 = load("tb2", [128, 4], _dt_f32)
        w3 = load("w3", [128, 4])
        tb3 = load("tb3", [1, 1], _dt_f32)

        eg0 = sb.tile([128, BW * G], _dt_f8, name="eg0")
        eg1 = sb.tile([128, BW * G], _dt_f8, name="eg1")
        egs = [eg0, eg1, eg0, eg1]
        zsb = sb.tile([128, BW * G], _dt_f8)
        zstk = sb.tile([128, 8 * TS], _dt_f8)
        nc.vector.memset(zstk[:], 0.0)

        # gpsimd copies run ~5x below DVE speed: keep all drains and the
        # scramble on vector/scalar only.
        drain_engines = [nc.vector, nc.scalar]
        scram_engines = [nc.vector, nc.scalar]

        def cp(engine, dst, src):
            if engine is nc.scalar:
                engine.copy(dst, src)
            else:
                engine.tensor_copy(dst, src)

        def phase_a(n, eg):
            """gather + transpose tile n into eg (col = 32*sample + t)."""
            ef = eg[:]
            for c in range(CH):
                C = CH * n + c
                idxt = db.tile([128, NE], _dt_i32, name="idxt")
                nc.sync.dma_start(idxt[:], t["idx"][128 * C:128 * (C + 1), :])
                esm = db.tile([128, NE * D], _dt_f8, name="esm")
                nc.gpsimd.indirect_dma_start(
                    out=esm[:], out_offset=None,
                    in_=t["tbl"][:],
                    in_offset=bass.IndirectOffsetOnAxis(ap=idxt[:], axis=0),
                )
                for t8 in range(4):
                    nt8 = 8 if t8 < 3 else 2
                    # fp8 PE transpose must write with element step 2
                    trp = trps.tile([128, 256 * 8], _dt_f8,
                                    name="trp", tag="trp")
                    tf = trp[:]
                    for k in range(nt8):
                        ti = 8 * t8 + k
                        tout = bass.AP(tf.tensor, tf.offset + 256 * k,
                                       [tf.ap[0], [2, 128]])
                        nc.tensor.transpose(
                            tout, esm[:, 128 * ti:128 * (ti + 1)], ident[:])
                    dst = bass.AP(ef.tensor,
                                  ef.offset + BW * 32 * c + (8 * t8 + 1),
                                  [ef.ap[0], [1, nt8], [32, 128]])
                    src = bass.AP(tf.tensor, tf.offset,
                                  [tf.ap[0], [256, nt8], [2, 128]])
                    eng = drain_engines[(c + t8) % 2]
                    cp(eng, dst, src)

        def phase_b(n, eg):
            """bottom MLP -> x into eg slot 0 (x*S0) and zstk blk 0 (x*SX)."""
            h0 = sp.tile([128, 4 * 512], _dt_bf16, name="h0")
            for m in range(4):
                ps = mmps.tile([128, 512], _dt_f32, name="mm", tag="mm")
                nc.tensor.matmul(ps[:], bw0[:, 128 * m:128 * (m + 1)],
                                 dxt[:, TS * n:TS * (n + 1)],
                                 start=True, stop=True)
                nc.scalar.activation(h0[:, 512 * m:512 * (m + 1)], ps[:],
                                     Relu, bias=bb0[:, m:m + 1])
            h1b = sp.tile([128, 2 * 512], _dt_bf16, name="h1b")
            for m in range(2):
                ps = mmps.tile([128, 512], _dt_f32, name="mm", tag="mm")
                for k in range(4):
                    nc.tensor.matmul(
                        ps[:], bw1[:, 256 * k + 128 * m:256 * k + 128 * (m + 1)],
                        h0[:, 512 * k:512 * (k + 1)],
                        start=(k == 0), stop=(k == 3))
                nc.scalar.activation(h1b[:, 512 * m:512 * (m + 1)], ps[:],
                                     Relu, bias=bb1[:, m:m + 1])
            ps = mmps.tile([128, 512], _dt_f32, name="mm", tag="mm")
            for k in range(2):
                nc.tensor.matmul(ps[:], bw2[:, 128 * k:128 * (k + 1)],
                                 h1b[:, 512 * k:512 * (k + 1)],
                                 start=(k == 0), stop=(k == 1))
            ef = eg[:]
            xdst = bass.AP(ef.tensor, ef.offset, [ef.ap[0], [32, 512]])
            nc.scalar.activation(xdst, ps[:], Relu, bias=bb2h[:, 0:1],
                                 scale=S0)
            nc.scalar.activation(zstk[:, 0:512], ps[:], Relu,
                                 bias=bb2x[:, 0:1], scale=SX)

        def phase_c(n, eg):
            """Gram matmuls into zsb (rows 32s+t, col-in-group 32s'+t')."""
            for r in range(G // 4):
                bank = grps.tile([128, 512], _dt_f32, name="grb", tag="gr")
                for k in range(4):
                    g = 4 * r + k
                    blk = eg[:, BW * g:BW * (g + 1)]
                    nc.tensor.matmul(bank[:, 128 * k:128 * (k + 1)],
                                     blk, blk, start=True, stop=True)
                eng = drain_engines[r % 2]
                cp(eng, zsb[:, 512 * r:512 * (r + 1)], bank[:])

        def phase_d(n):
            """merged scramble: zstk[32u+j, 512(q+1)+4g+s] = Z~[4q+u, j]."""
            for e, (u, s) in enumerate((u, s) for u in range(4)
                                       for s in range(4)):
                nq = 7 if u < 3 else 6   # i = 4q+u must stay < 27
                sl = zsb[32 * s:32 * s + NT, :]
                src = bass.AP(sl.tensor, sl.offset + 32 * s + u,
                              [sl.ap[0], [4, nq], [BW, G]])
                dl = zstk[32 * u:32 * u + NT, :]
                dst = bass.AP(dl.tensor, dl.offset + 512 + s,
                              [dl.ap[0], [512, nq], [4, G]])
                cp(scram_engines[e % 2], dst, src)

        def phase_e(n, eg):
            """top MLP on zstk = [16x ; Z~ k-tiles], fp8 DoubleRow."""
            zf = zstk[:]
            h1t = sp.tile([128, 8 * 512], _dt_f8, name="h1t")
            for m in range(8):
                ps = mmps.tile([128, 512], _dt_f32, name="mm", tag="mm")
                for p in range(4):
                    wap = bass.AP(wcat[:].tensor,
                                  wcat[:].offset + 2048 * p + 128 * m,
                                  [wcat[:].ap[0], [1024, 2], [1, 128]])
                    rap = bass.AP(zf.tensor, zf.offset + 1024 * p,
                                  [zf.ap[0], [512, 2], [1, 512]])
                    nc.tensor.matmul(ps[:], wap, rap,
                                     start=(p == 0), stop=(p == 3),
                                     perf_mode=DR)
                nc.scalar.activation(h1t[:, 512 * m:512 * (m + 1)], ps[:],
                                     Relu, bias=tb0[:, m:m + 1],
                                     scale=SH / SZ)
            h1f = h1t[:]
            h2t = sp.tile([128, 8 * 512], _dt_f8, name="h2t")
            for m in range(8):
                ps = mmps.tile([128, 512], _dt_f32, name="mm", tag="mm")
                for p in range(4):
                    wap = bass.AP(w1[:].tensor,
                                  w1[:].offset + 2048 * p + 128 * m,
                                  [w1[:].ap[0], [1024, 2], [1, 128]])
                    rap = bass.AP(h1f.tensor, h1f.offset + 1024 * p,
                                  [h1f.ap[0], [512, 2], [1, 512]])
                    nc.tensor.matmul(ps[:], wap, rap,
                                     start=(p == 0), stop=(p == 3),
                                     perf_mode=DR)
                nc.scalar.activation(h2t[:, 512 * m:512 * (m + 1)], ps[:],
                                     Relu, bias=tb1[:, m:m + 1],
                                     scale=SH / (SH * SH))
            h2f = h2t[:]
            h3t = sp.tile([128, 4 * 512], _dt_bf16, name="h3t")
            for m in range(4):
                ps = mmps.tile([128, 512], _dt_f32, name="mm", tag="mm")
                for p in range(4):
                    wap = bass.AP(w2[:].tensor,
                                  w2[:].offset + 1024 * p + 128 * m,
                                  [w2[:].ap[0], [512, 2], [1, 128]])
                    rap = bass.AP(h2f.tensor, h2f.offset + 1024 * p,
                                  [h2f.ap[0], [512, 2], [1, 512]])
                    nc.tensor.matmul(ps[:], wap, rap,
                                     start=(p == 0), stop=(p == 3),
                                     perf_mode=DR)
                nc.scalar.activation(h3t[:, 512 * m:512 * (m + 1)], ps[:],
                                     Relu, bias=tb2[:, m:m + 1],
                                     scale=1.0 / (SH * SH))
            ps3 = mmps.tile([128, 512], _dt_f32, name="mm", tag="mm")
            for k in range(4):
                nc.tensor.matmul(ps3[0:1, :], w3[:, k:k + 1],
                                 h3t[:, 512 * k:512 * (k + 1)],
                                 start=(k == 0), stop=(k == 3))
            outsb = db.tile([1, 512], _dt_f32, name="outsb")
            nc.scalar.activation(outsb[:], ps3[0:1, :], Sigmoid,
                                 bias=tb3[0:1, 0:1])
            nc.sync.dma_start(t["out"][n:n + 1, :], outsb[:])

        # software pipeline: A/B of tile n+1 slot between C/D and E of tile n
        phase_a(0, egs[0])
        phase_b(0, egs[0])
        for n in range(NTILES):
            phase_c(n, egs[n])
            phase_d(n)
            if n + 1 < NTILES:
                phase_a(n + 1, egs[n + 1])
                phase_b(n + 1, egs[n + 1])
            phase_e(n, egs[n])


def _build():
    if "nc" in _CACHE:
        return _CACHE["nc"]
    nc = bacc.Bacc("TRN2", target_bir_lowering=False, debug=False,
                   num_devices=N_CORES)
    t = {}

    def dram(name, shape, dt, kind="ExternalInput"):
        t[name] = nc.dram_tensor(name, shape, dt, kind=kind).ap()

    dram("tbl", [NE * VOCAB, D], _dt_f8)
    dram("idx", [SPC, NE], _dt_i32)
    dram("dxt", [16, SPC], _dt_bf16)
    dram("bw0", [16, 512], _dt_bf16)
    dram("bb0", [128, 4], _dt_f32)
    dram("bw1", [128, 4 * 256], _dt_bf16)
    dram("bb1", [128, 2], _dt_f32)
    dram("bw2", [128, 2 * 128], _dt_bf16)
    dram("bb2h", [128, 1], _dt_f32)
    dram("bb2x", [128, 1], _dt_f32)
    dram("wcat", [128, 8 * 1024], _dt_f8)
    dram("tb0", [128, 8], _dt_f32)
    dram("w1", [128, 8 * 1024], _dt_f8)
    dram("tb1", [128, 8], _dt_f32)
    dram("w2", [128, 8 * 512], _dt_f8)
    dram("tb2", [128, 4], _dt_f32)
    dram("w3", [128, 4], _dt_bf16)
    dram("tb3", [1, 1], _dt_f32)
    dram("out", [NTILES, TS], _dt_f32, kind="ExternalOutput")

    with tile.TileContext(nc) as tc:
        _emit(tc, t)
    nc.compile()

    _CACHE["nc"] = nc
    return nc


def _ktile(w, kt, m):
    """[K, M] -> [128, (K//128) * M] with column kt*M + mm = w[128*kt + p, mm]."""
    K, Mo = w.shape
    return np.ascontiguousarray(
        w.reshape(K // 128, 128, Mo).transpose(1, 0, 2).reshape(128, -1))


def _f8(a):
    return np.clip(a, -240.0, 240.0).astype(F8)


def _shared_inputs(inputs):
    emb = np.asarray(inputs["emb_tables"]).astype(F32)
    tbl = _f8(SE * emb.reshape(NE * VOCAB, D))

    sh = {"tbl": tbl}
    sh["bw0"] = np.zeros((16, 512), BF16)
    sh["bw0"][:13] = np.asarray(inputs["bot_W0"]).astype(BF16)
    sh["bb0"] = np.asarray(inputs["bot_b0"]).astype(F32).reshape(4, 128).T.copy()
    sh["bw1"] = _ktile(np.asarray(inputs["bot_W1"]).astype(BF16), 4, 256)
    sh["bb1"] = np.asarray(inputs["bot_b1"]).astype(F32).reshape(2, 128).T.copy()
    sh["bw2"] = _ktile(np.asarray(inputs["bot_W2"]).astype(BF16), 2, 128)
    b2 = np.asarray(inputs["bot_b2"]).astype(F32).reshape(1, 128).T.copy()
    sh["bb2h"] = S0 * b2
    sh["bb2x"] = SX * b2

    w0 = np.asarray(inputs["top_W0"]).astype(F32)
    svec = np.array([S0] + [SE] * NE, F32)
    wgrid = np.zeros((NT, NT, 1024), F32)
    wgrid[LI, LJ] = w0[128:479]
    wgrid *= (SZ / (svec[:, None] * svec[None, :]))[:, :, None]
    wz4 = np.zeros((7, 128, 1024), F32)
    for i in range(NT):
        q, u = i // 4, i % 4
        wz4[q, 32 * u:32 * u + NT] = wgrid[i]
    wall = np.concatenate([(SZ / SX) * w0[:128][None], wz4], axis=0)
    sh["wcat"] = _f8(np.ascontiguousarray(
        wall.transpose(1, 0, 2).reshape(128, 8 * 1024)))
    sh["tb0"] = SH * np.asarray(inputs["top_b0"]).astype(F32).reshape(8, 128).T
    sh["w1"] = _f8(_ktile(SH * np.asarray(inputs["top_W1"]).astype(F32), 8, 1024))
    sh["tb1"] = SH * np.asarray(inputs["top_b1"]).astype(F32).reshape(8, 128).T
    sh["w2"] = _f8(_ktile(SH * np.asarray(inputs["top_W2"]).astype(F32), 8, 512))
    sh["tb2"] = np.asarray(inputs["top_b2"]).astype(F32).reshape(4, 128).T.copy()
    sh["w3"] = _ktile(np.asarray(inputs["top_W3"]).astype(BF16), 4, 1)
    sh["tb3"] = np.asarray(inputs["top_b3"]).astype(F32).reshape(1, 1)
    return sh


def _in_maps(inputs):
    sh = _shared_inputs(inputs)
    idx = np.asarray(inputs["indices"]).astype(np.int64)      # [26, B]
    gidx = (idx + (np.arange(NE) * VOCAB)[:, None]).astype(np.int32)
    dx = np.asarray(inputs["dense_x"]).astype(F32)            # [B, 13]
    maps = []
    for core in range(N_CORES):
        sl = slice(SPC * core, SPC * (core + 1))
        m = dict(sh)
        m["idx"] = np.ascontiguousarray(gidx[:, sl].T)        # [2048, 26]
        dxt = np.zeros((16, SPC), BF16)
        dxt[:13] = dx[sl].T.astype(BF16)
        m["dxt"] = dxt
        maps.append(m)
    return maps


def _run(inputs, trace=False):
    nc = _build()
    maps = _in_maps(inputs)
    old_m = nc.m
    nc.m = _CACHE.setdefault("hwm", get_hw_module(nc.m))
    try:
        res = bass_utils.run_bass_kernel_spmd(
            nc, maps, core_ids=list(range(N_CORES)), trace=trace)
    finally:
        nc.m = old_m
    out = np.concatenate([r["out"].reshape(-1) for r in res.results])
    return out.astype(F32).reshape(B, 1), res


def kernel(**inputs):
    out, _ = _run(inputs, trace=False)
    return out
